# revision 3
# baseline (speedup 1.0000x reference)
"""Trainium2 Bass kernel for nn_Encoder_755914244431 (LSTM encoder).

B=128, T=512, F=256, H=512. Data-parallel: batch sharded 8x (16/core),
LSTM weights replicated. Returns (output [B,T,H], h_last [B,H], c_last [B,H]).

Per-core design:
 - Phase 1: xz = x @ W via PE GEMM (fp32) -> DRAM scratch [T, 16, 2048].
 - Phase 2 (recurrent loop, fully unrolled, per step):
     * z psum tile [128, 512] (ONE bank): gate q at partitions [32q, 32q+16),
       computed by 4-way col-tiled matmuls (tile_position=(0, 32q)) which the
       PE array runs concurrently (one col-group per 32-col strip):
         round 0: lhsT = [I16; ones] [17,16], rhs = [xz_t; bias] [17, 512] slice
                  (folds both the xz addend and the bias into the accumulation)
         rounds 1-4: lhsT = hT chunk [128,16], rhs = R K-chunk [128, 512] slice.
     * Gate order in weight columns is host-permuted to [i,f,o,g], so ONE
       sigmoid ACT covers psum partitions [0:80) (i@0, f@32, o@64) and one
       tanh covers g-cell @96. ACT ops read psum and write SBUF at shifted
       32-aligned partition bases to realign operands for the DVE.
     * DVE: t1 = f*c, t2 = i*g, c_new = t1+t2, h = o*tanh(c_new).
     * hT for the next step: 4 PE transposes ([16,128]->[128,16]) into one psum
       bank + one DVE copy -> SBUF [128, 64].
"""
import sys
sys.path.insert(0, "/opt/trn_rl_repo")
import numpy as np
import concourse.bacc as bacc
import concourse.mybir as mybir
import concourse.tile as tile
from concourse import bass_utils, masks

F32 = mybir.dt.float32
AF = mybir.ActivationFunctionType

_B_FULL, _T, _F, _H = 128, 512, 256, 512
_N_CORES = 8
_NC_CACHE = {}


def _build_nc(T=_T, B=16, F=_F, H=_H, n_cores=_N_CORES):
    G4 = 4 * H
    KF = F // 128
    KH = H // 128
    NG = G4 // 512
    TSZ = min(128, T)
    NTC = (T + TSZ - 1) // TSZ

    nc = bacc.Bacc("TRN2", target_bir_lowering=False, debug=False, num_devices=n_cores)
    x_d = nc.dram_tensor("x", [B, T, F], F32, kind="ExternalInput").ap()
    w_d = nc.dram_tensor("w", [F, G4], F32, kind="ExternalInput").ap()
    r_d = nc.dram_tensor("r", [H, G4], F32, kind="ExternalInput").ap()
    bias_d = nc.dram_tensor("bias", [1, G4], F32, kind="ExternalInput").ap()
    i16b_d = nc.dram_tensor("i16b", [B + 1, B], F32, kind="ExternalInput").ap()
    out_d = nc.dram_tensor("out", [B, T, H], F32, kind="ExternalOutput").ap()
    h_d = nc.dram_tensor("h_last", [B, H], F32, kind="ExternalOutput").ap()
    c_d = nc.dram_tensor("c_last", [B, H], F32, kind="ExternalOutput").ap()

    with tile.TileContext(nc) as tc:
        with (
            tc.tile_pool(name="const", bufs=1) as constp,
            tc.tile_pool(name="dram", bufs=1, space="DRAM") as dramp,
        ):
            xz_d = dramp.tile([T, B, G4], F32)

            ident = constp.tile([128, 128], F32)
            masks.make_identity(nc, ident[:])
            i16b = constp.tile([B + 1, B], F32)
            nc.sync.dma_start(i16b[:], i16b_d)
            bias_sb = constp.tile([1, G4], F32)
            nc.sync.dma_start(bias_sb[:], bias_d)
            r_sb = constp.tile([128, KH, G4], F32)
            nc.sync.dma_start(r_sb[:], r_d.rearrange("(k p) n -> p k n", p=128))
            w_sb = constp.tile([128, KF, G4], F32)
            nc.sync.dma_start(w_sb[:], w_d.rearrange("(k p) n -> p k n", p=128))
            zero64 = constp.tile([128, KH * B], F32)
            nc.vector.memset(zero64[:], 0.0)

            # ---------------- Phase 1: xz = x @ W -> xz_d [T, B, 2048] ------------
            with (
                tc.tile_pool(name="p1x", bufs=3) as p1x,
                tc.tile_pool(name="p1xT", bufs=3) as p1xT,
                tc.tile_pool(name="p1ps", bufs=2, space="PSUM") as p1ps,
                tc.tile_pool(name="p1psT", bufs=2, space="PSUM") as p1psT,
                tc.tile_pool(name="p1o", bufs=3) as p1o,
            ):
                for b in range(B):
                    for tc_i in range(NTC):
                        t0 = tc_i * TSZ
                        x_t = p1x.tile([TSZ, F], F32, tag="x")
                        nc.sync.dma_start(x_t[:], x_d[b, t0:t0 + TSZ, :])
                        xT_ps = p1psT.tile([128, KF, TSZ], F32, tag="xT_ps")
                        for k in range(KF):
                            nc.tensor.matmul(xT_ps[:, k, :], x_t[:, 128 * k:128 * (k + 1)],
                                             ident[:TSZ, :TSZ], is_transpose=True,
                                             start=(k == 0), stop=(k == KF - 1),
                                             skip_group_check=True)
                        xT = p1xT.tile([128, KF, TSZ], F32, tag="xT")
                        nc.vector.tensor_copy(xT[:], xT_ps[:])
                        for n in range(NG):
                            zps = p1ps.tile([TSZ, 512], F32, tag="zps")
                            for k in range(KF):
                                nc.tensor.matmul(zps[:], xT[:, k, :],
                                                 w_sb[:, k, 512 * n:512 * (n + 1)],
                                                 start=(k == 0), stop=(k == KF - 1))
                            o_t = p1o.tile([TSZ, 512], F32, tag="o")
                            if n % 2 == 0:
                                nc.scalar.copy(o_t[:], zps[:])
                            else:
                                nc.vector.tensor_copy(o_t[:], zps[:])
                            nc.sync.dma_start(xz_d[t0:t0 + TSZ, b, 512 * n:512 * (n + 1)], o_t[:])

            # ---------------- Phase 2: recurrent loop -----------------------------
            with (
                tc.tile_pool(name="xzp", bufs=4) as xzp,
                tc.tile_pool(name="zps", bufs=1, space="PSUM") as zpsp,
                tc.tile_pool(name="hTps", bufs=2, space="PSUM") as hTpsp,
                tc.tile_pool(name="gat", bufs=2) as gatp,
                tc.tile_pool(name="cst", bufs=2) as cstp,
                tc.tile_pool(name="hp", bufs=2) as hp,
                tc.tile_pool(name="hTp", bufs=2) as hTp,
            ):
                z = zpsp.tile([128, 512], F32, tag="z")
                nc.vector.memset(z[:], 0.0)

                hT_sb = hTp.tile([128, KH * B], F32, tag="hT")
                nc.vector.tensor_copy(hT_sb[:], zero64[:])
                c_tile = cstp.tile([48, H], F32, tag="c")
                nc.vector.memset(c_tile[32:48, :], 0.0)

                for t in range(T):
                    xz_t = xzp.tile([B + 1, G4], F32, tag="xz")
                    nc.sync.dma_start(xz_t[:B, :], xz_d[t, :, :])
                    nc.sync.dma_start(xz_t[B:B + 1, :], bias_sb[:])

                    for q in range(NG):
                        nc.tensor.matmul(z[32 * q:32 * q + B, :], i16b[:],
                                         xz_t[:, 512 * q:512 * (q + 1)],
                                         start=True, stop=False, tile_position=(0, 32 * q),
                                         skip_group_check=True)
                    for k in range(KH):
                        for q in range(NG):
                            nc.tensor.matmul(z[32 * q:32 * q + B, :],
                                             hT_sb[:, 16 * k:16 * (k + 1)],
                                             r_sb[:, k, 512 * q:512 * (q + 1)],
                                             start=False, stop=(k == KH - 1),
                                             tile_position=(0, 32 * q),
                                             skip_group_check=True)

                    gates = gatp.tile([80, H], F32, tag="gates")
                    nc.scalar.activation(gates[:], z[0:80, :], AF.Sigmoid)
                    g_sb = gatp.tile([B, H], F32, tag="g")
                    nc.scalar.activation(g_sb[:], z[96:96 + B, :], AF.Tanh)

                    t1 = gatp.tile([B, H], F32, tag="t1")
                    nc.vector.tensor_mul(t1[:], gates[32:48, :], c_tile[32:48, :])
                    t2 = gatp.tile([B, H], F32, tag="t2")
                    nc.vector.tensor_mul(t2[:], gates[0:16, :], g_sb[:])
                    c_new = cstp.tile([48, H], F32, tag="c")
                    nc.vector.tensor_add(c_new[32:48, :], t1[:], t2[:])
                    th = gatp.tile([80, H], F32, tag="th")
                    nc.scalar.activation(th[64:80, :], c_new[32:48, :], AF.Tanh)
                    h_sb = hp.tile([B, H], F32, tag="h")
                    nc.vector.tensor_mul(h_sb[:], gates[64:80, :], th[64:80, :])

                    nc.sync.dma_start(out_d[:, t, :], h_sb[:])

                    if t == T - 1:
                        nc.sync.dma_start(h_d[:], h_sb[:])
                        nc.sync.dma_start(c_d[:], c_new[32:48, :])
                        break

                    hT_ps = hTpsp.tile([128, KH * B], F32, tag="hT_ps")
                    for k in range(KH):
                        nc.tensor.matmul(hT_ps[:, 16 * k:16 * (k + 1)],
                                         h_sb[:, 128 * k:128 * (k + 1)], ident[:B, :B],
                                         is_transpose=True, start=(k == 0), stop=(k == KH - 1),
                                         skip_group_check=True)
                    hT_sb = hTp.tile([128, KH * B], F32, tag="hT")
                    nc.vector.tensor_copy(hT_sb[:], hT_ps[:])
                    c_tile = c_new
    nc.compile()
    return nc


def _host_inputs(x_full, kernel_w, recurrent_kernel, bias, n_cores=_N_CORES):
    B_full = x_full.shape[0]
    Bs = B_full // n_cores

    def perm(w):  # [*, 4H] columns -> gate order [i, f, o, g]
        i, f, g, o = np.split(np.asarray(w, np.float32), 4, axis=-1)
        return np.ascontiguousarray(np.concatenate([i, f, o, g], axis=-1))

    w_p = perm(kernel_w)
    r_p = perm(recurrent_kernel)
    b_p = perm(np.asarray(bias, np.float32).reshape(1, -1))
    i16b = np.concatenate([np.eye(Bs, dtype=np.float32), np.ones((1, Bs), np.float32)], 0)
    x_np = np.ascontiguousarray(np.asarray(x_full, np.float32))
    return [
        {"x": np.ascontiguousarray(x_np[c * Bs:(c + 1) * Bs]), "w": w_p, "r": r_p,
         "bias": b_p, "i16b": i16b}
        for c in range(n_cores)
    ]


def kernel(x, kernel, recurrent_kernel, bias):
    """Full-input LSTM forward on 8 NeuronCores. Returns (output, h_last, c_last)."""
    x = np.asarray(x, np.float32)
    B_full, T, F = x.shape
    key = (B_full, T, F)
    if key not in _NC_CACHE:
        _NC_CACHE[key] = _build_nc(T=T, B=B_full // _N_CORES, F=F,
                                   H=np.asarray(recurrent_kernel).shape[0])
    nc = _NC_CACHE[key]
    in_maps = _host_inputs(x, kernel, recurrent_kernel, bias)
    res = bass_utils.run_bass_kernel_spmd(nc, in_maps, core_ids=list(range(_N_CORES)))
    out = np.concatenate([res.results[c]["out"] for c in range(_N_CORES)], 0)
    h = np.concatenate([res.results[c]["h_last"] for c in range(_N_CORES)], 0)
    c = np.concatenate([res.results[c]["c_last"] for c in range(_N_CORES)], 0)
    return out, h, c
